# revision 22
# baseline (speedup 1.0000x reference)
"""Trainium2 Bass kernel for nn_ConvSelfAttentionModule (B=4, C=256, H=W=64).

Reference computation per image (xf = x reshaped to [C, N], N = H*W = 4096):
    q = wq @ xf + bq                       [128, N]
    k = wk @ xf + bk                       [128, N]
    v = wv @ xf + bv                       [256, N]
    s[m, n]   = sum_d q[d, m] k[d, n]      [N, N]
    attn      = softmax_n(s)
    af[c, n]  = sum_m v[c, m] attn[m, n]   [256, N]
    out = gamma * af + x

Sharding: 8 cores = 4 images x 2 m-chunks of M=2048 rows of the attention
matrix. Core (b, half) computes af_part[c, n] = sum_{m in chunk} v'[c, m] E[m, n]
for all n, where E = exp(s - 20) and v'[c, m] = gamma * v[c, m] / rowsum_E[m]
(softmax normalizer and gamma folded into v). The host adds the two partials
per image and adds x. Each core gets its image's xf pre-rolled by -m0 columns
so the kernel is SPMD-identical; the host rolls the partial back.

Single-sweep schedule (the key difference from the two-sweep version): each
m-tile's four 1024-wide score slabs (2 per n-half) are computed and exp'd
back-to-back, with the rowsum accumulated by ACT (accum_out) on every slab,
so v'_mt is final immediately after the tile's last slab. The af matmuls are
then interleaved into the SAME sweep in windows of 4 m-tiles: while ACT exps
m-tile t, the PE runs af matmuls of the previous window, keeping both engines
dense. PSUM: 2x [128,1024] score slabs (4 banks) + 2x [128,1024] af blocks
(4 banks). af partials accumulate across windows in SBUF (bf16) via DVE adds;
output is stored bf16 (host adds partials + x in fp32).
"""

import numpy as np
import ml_dtypes

import concourse.bass as bass  # noqa: F401  (bass types via bacc/tile)
import concourse.tile as tile
from concourse import bacc, mybir
from concourse.bass_utils import run_bass_kernel_spmd

dt = mybir.dt

P = 128          # partitions / q,k channel dim
C = 256          # channels
N = 4096         # pixels per image
M = 2048         # per-core m-chunk
MT = M // P      # 16 m-tiles
B = 4
N_CORES = 8
EXP_SHIFT = -20.0  # constant subtracted inside exp; cancels in softmax
# af windows as (first m-tile, n m-tiles); last window kept small so the
# post-sweep PE tail is short.
WINDOWS = [(0, 4), (4, 4), (8, 4), (12, 2), (14, 2)]
NWIN = len(WINDOWS)

_CACHE = {}


def build_nc():
    nc = bacc.Bacc("TRN2", target_bir_lowering=False, debug=False,
                   num_devices=N_CORES)
    f32, f32r, bf16 = dt.float32, dt.float32r, dt.bfloat16
    AF = mybir.ActivationFunctionType

    x = nc.dram_tensor("x", [C, N], bf16, kind="ExternalInput").ap()
    # All weights/biases packed into one [128, 1282] bf16 tensor so they land
    # in ONE DMA with 2.5KB-per-partition descriptors (separate [128,128]
    # tiles were descriptor-bound: ~305B packets, 11us to arrive).
    # Columns: [wk0 128][wk1 128][wq0 128][wq1 128][wv0 256][wv1 256]
    #          [bv_bc 256]
    wpk = nc.dram_tensor("wpack", [P, 1280], bf16, kind="ExternalInput").ap()
    bpk = nc.dram_tensor("bpack", [P, 2], f32, kind="ExternalInput").ap()
    out = nc.dram_tensor("out_part", [C, N], bf16, kind="ExternalOutput").ap()
    out2 = nc.dram_tensor("out_part2", [P, 4, 1024], bf16,
                          kind="ExternalOutput").ap()

    with tile.TileContext(nc) as tc:
        with (
            tc.tile_pool(name="consts", bufs=1) as consts,
            tc.tile_pool(name="xs", bufs=8) as xs,
            tc.tile_pool(name="big", bufs=1) as big,
            tc.tile_pool(name="es", bufs=20) as es,
            tc.tile_pool(name="pss", bufs=2, space="PSUM") as pss,
            tc.tile_pool(name="psa", bufs=2, space="PSUM") as psa,
        ):
            # ---- constants ----
            # Dummy exp first so the ACT table load (~2.7us) happens before
            # anything else on the ACT queue; then the weights ride the
            # scalar-engine HWDGE ring (fast, parallel to the sync ring that
            # carries x) instead of the slow gpsimd SWDGE path.
            shift_t = consts.tile([P, 1], f32, name="shift_t", tag="shift_t")
            nc.vector.memset(shift_t, EXP_SHIFT)
            warm_t = consts.tile([P, 1], f32, name="warm_t", tag="warm_t")
            nc.scalar.activation(warm_t, shift_t, AF.Exp, bias=shift_t[:, 0:1],
                                 scale=1.0)

            wp = consts.tile([P, 1280], bf16, name="wp", tag="wp")
            # split the pack: wk+wq (512 cols) land first so the k/q matmuls
            # don't wait for the whole pack
            nc.scalar.dma_start(out=wp[:, 0:512], in_=wpk[:, 0:512])
            bp = consts.tile([P, 2], f32, name="bp", tag="bp")
            nc.scalar.dma_start(out=bp, in_=bpk)
            nc.scalar.dma_start(out=wp[:, 512:1280], in_=wpk[:, 512:1280])
            wk_t = [wp[:, 0:128], wp[:, 128:256]]
            wq_t = [wp[:, 256:384], wp[:, 384:512]]
            wv_t = [wp[:, 512:768], wp[:, 768:1024]]
            bv_bc = wp[:, 1024:1280]
            bk_t = bp[:, 0:1]
            bq_t = bp[:, 1:2]

            rs = consts.tile([P, MT, 4], f32, name="rs", tag="rs")
            rr = consts.tile([P, MT], f32, name="rr", tag="rr")

            # ---- x streamed in 4 column-chunks of 1024 ----
            xg = []
            for g in range(4):
                x0 = xs.tile([P, 1024], bf16, name=f"xg0_{g}", tag="xg")
                x1 = xs.tile([P, 1024], bf16, name=f"xg1_{g}", tag="xg")
                gsl = slice(g * 1024, (g + 1) * 1024)
                nc.sync.dma_start(out=x0, in_=x[0:P, gsl])
                nc.sync.dma_start(out=x1, in_=x[P:C, gsl])
                xg.append((x0, x1))

            k_sb = big.tile([P, N], bf16, name="k_sb", tag="k_sb")
            q_sb = big.tile([P, M], bf16, name="q_sb", tag="q_sb")
            v_sb = big.tile([P, MT, C], bf16, name="v_sb", tag="v_sb")
            af_sb = big.tile([P, 8, 1024], bf16, name="af_sb", tag="af_sb")
            stage = big.tile([P, 4, 1024], bf16, name="stage", tag="stage")

            def k_chunk(g):
                x0, x1 = xg[g]
                kp = psa.tile([P, 1024], f32, name=f"kp{g}", tag="pa")
                for j in range(2):
                    sl = slice(j * 512, (j + 1) * 512)
                    nc.tensor.matmul(kp[:, sl], wk_t[0], x0[:, sl],
                                     start=True, stop=False)
                    nc.tensor.matmul(kp[:, sl], wk_t[1], x1[:, sl],
                                     start=False, stop=True)
                nc.vector.tensor_scalar_add(k_sb[:, g * 1024:(g + 1) * 1024],
                                            kp, bk_t[:, 0:1])

            def q_chunk(g):
                x0, x1 = xg[g]
                qp = psa.tile([P, 1024], f32, name=f"qp{g}", tag="pa")
                for j in range(2):
                    sl = slice(j * 512, (j + 1) * 512)
                    nc.tensor.matmul(qp[:, sl], wq_t[0], x0[:, sl],
                                     start=True, stop=False)
                    nc.tensor.matmul(qp[:, sl], wq_t[1], x1[:, sl],
                                     start=False, stop=True)
                nc.vector.tensor_scalar_add(q_sb[:, g * 1024:(g + 1) * 1024],
                                            qp, bq_t[:, 0:1])

            def v_chunk(g, sub):
                # 4 m-tiles of vT (m-tiles 8g+4*sub .. +3)
                x0, x1 = xg[g]
                vp = psa.tile([P, 4, C], f32, name=f"vp{g}{sub}", tag="pa")
                for i in range(4):
                    t = sub * 4 + i
                    xsl = slice(t * P, (t + 1) * P)
                    nc.tensor.matmul(vp[:, i], x0[:, xsl], wv_t[0],
                                     start=True, stop=False)
                    nc.tensor.matmul(vp[:, i], x1[:, xsl], wv_t[1],
                                     start=False, stop=True)
                for i in range(4):
                    t = g * 8 + sub * 4 + i
                    nc.vector.tensor_add(v_sb[:, t, :], vp[:, i], bv_bc)

            e_tiles = {}

            def scores_slab(mt, h, s):
                # one [128,1024] slab of scores -> exp -> E, rowsum accum
                if (mt, h) not in e_tiles:
                    e_tiles[(mt, h)] = es.tile([P, 2048], bf16,
                                               name=f"e{mt}_{h}", tag="e")
                e_t = e_tiles[(mt, h)]
                sp = pss.tile([P, 1024], f32, name=f"sp{mt}{h}{s}", tag="ps")
                q_l = q_sb[:, mt * P:(mt + 1) * P]
                base = h * 2048 + s * 1024
                for j in range(2):
                    nc.tensor.matmul(sp[:, j * 512:(j + 1) * 512], q_l,
                                     k_sb[:, base + j * 512:base + (j + 1) * 512],
                                     start=True, stop=True)
                nc.scalar.activation(e_t[:, s * 1024:(s + 1) * 1024], sp,
                                     AF.Exp, bias=shift_t[:, 0:1], scale=1.0,
                                     accum_out=rs[:, mt, h * 2 + s:h * 2 + s + 1])

            def rs_chain(mt):
                nc.vector.reduce_sum(rr[:, mt:mt + 1], rs[:, mt, :],
                                     axis=mybir.AxisListType.X)
                nc.vector.reciprocal(rr[:, mt:mt + 1], rr[:, mt:mt + 1])
                nc.vector.tensor_scalar_mul(v_sb[:, mt, :], v_sb[:, mt, :],
                                            rr[:, mt:mt + 1])

            def af_unit(wi, b):
                # one af block (h, c, nq): accumulate window wi's m-tiles
                h, cc, nq = b >> 2, (b >> 1) & 1, b & 1
                w0, wl = WINDOWS[wi]
                ap = psa.tile([P, 1024], f32, name=f"af{wi}_{b}", tag="pa")
                for i, mt in enumerate(range(w0, w0 + wl)):
                    lhs = v_sb[:, mt, cc * P:(cc + 1) * P]
                    e_t = e_tiles[(mt, h)]
                    for j in range(2):
                        nc.tensor.matmul(
                            ap[:, j * 512:(j + 1) * 512], lhs,
                            e_t[:, nq * 1024 + j * 512:nq * 1024 + (j + 1) * 512],
                            start=(i == 0), stop=(i == wl - 1))
                dst = af_sb[:, b, :]
                osl = out[cc * P:(cc + 1) * P,
                          h * 2048 + nq * 1024:h * 2048 + (nq + 1) * 1024]
                if wi == 0:
                    nc.vector.tensor_copy(dst, ap)
                elif wi < NWIN - 1:
                    nc.vector.tensor_add(dst, ap, dst)
                    if wi == NWIN - 2 and b >= 4:
                        # blocks 4-7 finish via ACT+host: ship the
                        # through-window-(NWIN-2) sum now
                        nc.sync.dma_start(out=osl, in_=dst)
                else:
                    if b < 4:
                        # final combine on DVE
                        nc.vector.tensor_add(dst, ap, dst)
                        nc.sync.dma_start(out=osl, in_=dst)
                    else:
                        # final window partial via idle ACT; host adds it
                        st = stage[:, b - 4, :]
                        nc.scalar.copy(st, ap)
                        nc.sync.dma_start(out=out2[:, b - 4, :], in_=st)

            # schedule window wi's 8 af units across the (mt, h) emission
            # points of window wi+1
            unit_sched = {}
            for wi in range(NWIN - 1):
                nstart, nlen = WINDOWS[wi + 1]
                pts = [(mt, h) for mt in range(nstart, nstart + nlen)
                       for h in range(2)]
                per, rem = divmod(8, len(pts))
                idx = 0
                for pi, pt in enumerate(pts):
                    for _ in range(per + (1 if pi < rem else 0)):
                        unit_sched.setdefault(pt, []).append((wi, idx))
                        idx += 1

            # ---- emission ----
            # prologue: chunks 0,1 give k[0:2048] + all of q; the v
            # projections go LAST so their wait on wv never head-of-line
            # blocks the score matmuls in the PE FIFO.
            k_chunk(0); q_chunk(0)
            scores_slab(0, 0, 0)
            scores_slab(1, 0, 0)
            k_chunk(1); q_chunk(1)
            scores_slab(0, 0, 1)
            scores_slab(1, 0, 1)
            k_chunk(2)
            scores_slab(0, 1, 0)
            scores_slab(1, 1, 0)
            k_chunk(3)
            scores_slab(0, 1, 1)
            v_chunk(0, 0)
            rs_chain(0)
            scores_slab(1, 1, 1)
            v_chunk(0, 1)
            rs_chain(1)

            # steady-state sweep; previous window's af units interleaved
            for mt in range(2, MT):
                for h in range(2):
                    for s in range(2):
                        scores_slab(mt, h, s)
                    for wi, b in unit_sched.get((mt, h), ()):
                        af_unit(wi, b)
                if mt == 2:
                    v_chunk(1, 0)
                elif mt == 3:
                    v_chunk(1, 1)
                rs_chain(mt)

            # tail: last window's af
            for b in range(8):
                af_unit(NWIN - 1, b)

    nc.compile()
    return nc


def _get_nc():
    if "nc" not in _CACHE:
        _CACHE["nc"] = build_nc()
    return _CACHE["nc"]


def build_in_maps(x, wq, bq, wk, bk, wv, bv, gamma):
    bf = ml_dtypes.bfloat16
    x = np.asarray(x, np.float32)
    g = float(np.asarray(gamma).reshape(-1)[0])
    wqT = np.asarray(wq, np.float32).T
    wkT = np.asarray(wk, np.float32).T
    wvT = (g * np.asarray(wv, np.float32)).T
    bk2 = np.asarray(bk, np.float32).reshape(P, 1)
    bq2 = np.asarray(bq, np.float32).reshape(P, 1)
    bv_bc = np.broadcast_to((g * np.asarray(bv, np.float32)).reshape(1, C),
                            (P, C))
    wpack = np.ascontiguousarray(np.concatenate(
        [wkT[0:P, :], wkT[P:C, :], wqT[0:P, :], wqT[P:C, :],
         wvT[0:P, :], wvT[P:C, :], bv_bc], axis=1).astype(bf))
    bpack = np.ascontiguousarray(np.concatenate([bk2, bq2], axis=1))
    xf = x.reshape(B, C, N).astype(bf)
    in_maps = []
    for core in range(N_CORES):
        b, half = core // 2, core % 2
        xc = xf[b] if half == 0 else np.roll(xf[b], -M, axis=1)
        in_maps.append(dict(x=np.ascontiguousarray(xc), wpack=wpack,
                            bpack=bpack))
    return in_maps


def assemble(results, x):
    x = np.asarray(x, np.float32)
    af = np.zeros((B, C, N), np.float32)
    for core in range(N_CORES):
        b, half = core // 2, core % 2
        part = np.asarray(results[core]["out_part"]).astype(np.float32)
        part2 = np.asarray(results[core]["out_part2"]).astype(np.float32)
        # blocks 4-7 (h=1) got their last af window shipped separately
        for j in range(4):
            cc, nq = (j >> 1) & 1, j & 1
            part[cc * P:(cc + 1) * P,
                 2048 + nq * 1024:2048 + (nq + 1) * 1024] += part2[:, j, :]
        af[b] += part if half == 0 else np.roll(part, M, axis=1)
    return (af.reshape(x.shape) + x).astype(np.float32)


def kernel(x, wq, bq, wk, bk, wv, bv, gamma):
    nc = _get_nc()
    in_maps = build_in_maps(x, wq, bq, wk, bk, wv, bv, gamma)
    res = run_bass_kernel_spmd(nc, in_maps, core_ids=list(range(N_CORES)))
    return assemble(res.results, x)
